# revision 17
# baseline (speedup 1.0000x reference)
"""Trainium2 Bass kernel for bidirectional-NNF patch voting (bds_vote + blend).

Algorithm (mathematically identical to the reference):
  - wr == 2*ws exactly, so guide = (S1 + 2*S2) / (c1 + 2*c2), where S1/c1 are
    the pass-1 (src->ref) vote sum/count and S2/c2 the pass-2 (ref->src) ones.
  - All gather/scatter indexing is channel-independent -> precomputed on host.
  - Both voting passes become per-target gather lists; pass-2 collisions are
    layered (k-th contribution per target).  Each contribution gathers one
    2KB pixel vector (all 512 channels, pixel-major layout).
  - 8 cores each own 4608 target pixels (full channel dim).  Per core the
    distinct gather sources fit in <32K rows, so tables are compacted to make
    indices fit int16 for the GPSIMD dma_gather (InstDMAGatherAnt) primitive.
  - Device: ~90 dma_gather instructions (<=1024 idxs each, descriptor-ring
    limit) -> staging tiles -> DVE accumulate (x1 for pass-1, x2 for pass-2).
    response/min-max via DVE+GPSIMD reduces + a 2-float AllReduce; blend on DVE.
"""
import numpy as np
import os
import sys
import types

sys.path.insert(0, "/opt/trn_rl_repo")


def _knob(name):
    return os.environ.get("BNNF_" + name, "") not in ("", "0")

C, H, W = 512, 192, 192
N = H * W
ALPHA = 0.8
TAU = 0.05
PAD = -1
NCORES = 8
TPC = N // NCORES          # targets per core = 4608
SLOTS = TPC // 128         # acc slots per partition = 36
MAXIDX = 1024              # max idxs per dma_gather (descriptor-ring limit)
CHUNK = 6                  # blend/response chunk: slots per chunk

_D = {}  # module cache for the compiled program


def _build_pass1_planes(nnf_sr):
    ry = nnf_sr[..., 0].astype(np.int64)
    rx = nnf_sr[..., 1].astype(np.int64)
    planes = np.full((9, N), PAD, np.int32)
    k = 0
    for dy in (-1, 0, 1):
        for dx in (-1, 0, 1):
            gy = ry + dy
            gx = rx + dx
            valid = (gy >= 0) & (gy < H) & (gx >= 0) & (gx < W)
            src = np.where(valid, gy * W + gx, PAD).astype(np.int32)
            plane = np.full((H, W), PAD, np.int32)
            ty0, ty1 = max(dy, 0), H + min(dy, 0)
            tx0, tx1 = max(dx, 0), W + min(dx, 0)
            plane[ty0:ty1, tx0:tx1] = src[ty0 - dy:ty1 - dy, tx0 - dx:tx1 - dx]
            planes[k] = plane.ravel()
            k += 1
    return planes


def _build_pass2_planes(nnf_rs):
    sy = nnf_rs[..., 0].astype(np.int64)
    sx = nnf_rs[..., 1].astype(np.int64)
    tgt_all, src_all = [], []
    ryg, rxg = np.meshgrid(np.arange(H), np.arange(W), indexing="ij")
    for dy in (-1, 0, 1):
        for dx in (-1, 0, 1):
            ty = sy + dy
            tx = sx + dx
            gy = ryg + dy
            gx = rxg + dx
            valid = ((ty >= 0) & (ty < H) & (tx >= 0) & (tx < W) &
                     (gy >= 0) & (gy < H) & (gx >= 0) & (gx < W))
            tgt_all.append((ty * W + tx)[valid])
            src_all.append((gy * W + gx)[valid])
    tgt = np.concatenate(tgt_all)
    src = np.concatenate(src_all)
    order = np.argsort(tgt, kind="stable")
    tgt_s, src_s = tgt[order], src[order]
    counts = np.bincount(tgt_s, minlength=N)
    starts = np.concatenate(([0], np.cumsum(counts)[:-1]))
    rank = np.arange(len(tgt_s)) - starts[tgt_s]
    K2 = int(counts.max())
    planes = np.full((K2, N), PAD, np.int32)
    planes[rank, tgt_s] = src_s
    return planes, counts


def _wrap_idx(ix):
    """[n] int -> [128, n//16] int16 (wrapped in 16 partitions, replicated x8)."""
    return np.tile(ix.astype(np.int16).reshape(-1, 16).T, (8, 1))


def _prep(ref, nnf_sr, nnf_rs, f_a):
    """Host preprocessing. Returns (plan, per-core input maps skeleton)."""
    idxA = _build_pass1_planes(np.asarray(nnf_sr))
    idxB, c2 = _build_pass2_planes(np.asarray(nnf_rs))
    K2 = idxB.shape[0]

    refT = np.ascontiguousarray(np.asarray(ref).reshape(C, N).T)     # [N, C]
    faT = np.asarray(f_a).reshape(C, N).T                            # [N, C] (view)

    gorder = np.argsort(-c2, kind="stable")         # global rank -> target
    cores = []
    for c in range(NCORES):
        glob = gorder[c::NCORES]                    # local rank -> global target
        c2c = c2[glob]                              # descending by construction
        n_k = np.array([(c2c > k).sum() for k in range(K2)], np.int64)

        srcA = idxA[:, glob]                        # [9, TPC] global rows or PAD
        srcB = idxB[:, glob]                        # [K2, TPC]

        va = srcA[srcA != PAD]
        uniqA, invA = np.unique(va, return_inverse=True)
        remA = np.full(srcA.shape, PAD, np.int32)
        remA[srcA != PAD] = invA.astype(np.int32)

        vb = srcB[srcB != PAD]
        uniqB, invB = np.unique(vb, return_inverse=True)
        remB = np.full(srcB.shape, PAD, np.int32)
        remB[srcB != PAD] = invB.astype(np.int32)
        # pads point at a zeros row appended to each table (index = n_uniq):
        # gathers write zeros -> accumulate is a no-op, and we avoid the
        # GatherAnt negative-index tail path which wedges the device.
        remA[remA == PAD] = len(uniqA)
        remB[remB == PAD] = len(uniqB)

        cores.append(dict(glob=glob, n_k=n_k, remA=remA, remB=remB,
                          uniqA=uniqA, uniqB=uniqB))

    VA = max(len(cc["uniqA"]) for cc in cores) + 1
    VB = max(len(cc["uniqB"]) for cc in cores) + 1
    assert VA <= 32767 and VB <= 32767, (VA, VB)
    n_k_max = np.max([cc["n_k"] for cc in cores], axis=0)            # [K2]

    # instruction plan shared by all cores: (series, slot_off, nslots)
    instrs = []
    for k in range(9):                      # series 1: full slabs
        for s0 in range(0, SLOTS, 8):
            ns = min(8, SLOTS - s0)
            instrs.append((1, k, s0, ns))
    for k in range(K2):                     # series 2: prefix slabs
        sk = int(np.ceil(n_k_max[k] / 128))
        if sk == 0:
            continue
        for s0 in range(0, sk, 8):
            ns = min(8, sk - s0)
            instrs.append((2, k, s0, ns))

    # per-core idx blobs + tables
    in_maps = []
    for cc in cores:
        blobs = []
        for (series, k, s0, ns) in instrs:
            rem = cc["remA"] if series == 1 else cc["remB"]
            zrow = len(cc["uniqA"]) if series == 1 else len(cc["uniqB"])
            rr = np.arange(s0 * 128, (s0 + ns) * 128)
            ix = rem[k][rr] if k < rem.shape[0] else np.full(ns * 128, zrow, np.int32)
            blobs.append(_wrap_idx(ix))
        idx_blob = np.ascontiguousarray(np.concatenate(blobs, axis=1))

        TA = np.zeros((VA, C), np.float32)
        TA[:len(cc["uniqA"])] = refT[cc["uniqA"]]
        TB = np.zeros((VB, C), np.float32)
        TB[:len(cc["uniqB"])] = refT[cc["uniqB"]]

        fa_core = faT[cc["glob"]]                                    # [TPC, C]
        fa_blob = np.ascontiguousarray(
            fa_core.reshape(SLOTS, 128, C).transpose(1, 0, 2)).reshape(128, SLOTS * C)

        c1 = (cc["remA"] != len(cc["uniqA"])).sum(axis=0)
        c2r = (cc["remB"] != len(cc["uniqB"])).sum(axis=0)
        den = (c1 + 2 * c2r).astype(np.float32)
        winv = np.where(den == 0, np.float32(1.0), 1.0 / np.maximum(den, 1)).astype(np.float32)
        winv_blob = np.ascontiguousarray(winv.reshape(SLOTS, 128).T)  # [128, SLOTS]

        in_maps.append({"ta": TA, "tb": TB, "idx": idx_blob.astype(np.int16),
                        "fa": fa_blob.astype(np.float32), "winv": winv_blob})

    plan = dict(instrs=instrs, VA=VA, VB=VB,
                W_total=sum(ns * 8 for (_, _, _, ns) in instrs))
    return plan, in_maps, cores


def _build_program(plan):
    from concourse import bacc, bass, mybir, tile

    VA, VB = plan["VA"], plan["VB"]
    WT = plan["W_total"]
    nc = bacc.Bacc("TRN2", target_bir_lowering=False, debug=False,
                   num_devices=NCORES)
    dt = mybir.dt
    ta = nc.dram_tensor("ta", [VA, C], dt.float32, kind="ExternalInput").ap()
    tb = nc.dram_tensor("tb", [VB, C], dt.float32, kind="ExternalInput").ap()
    idx = nc.dram_tensor("idx", [128, WT], dt.int16, kind="ExternalInput").ap()
    fa = nc.dram_tensor("fa", [128, SLOTS * C], dt.float32, kind="ExternalInput").ap()
    winv = nc.dram_tensor("winv", [128, SLOTS], dt.float32, kind="ExternalInput").ap()
    out = nc.dram_tensor("out", [128, SLOTS * C], dt.float32, kind="ExternalOutput").ap()

    with tile.TileContext(nc) as tc:
        with tc.tile_pool(name="sbuf", bufs=1) as pool, \
             tc.tile_pool(name="stg", bufs=3) as stp, \
             tc.tile_pool(name="fac", bufs=2) as fap, \
             tc.tile_pool(name="dram", bufs=1, space="DRAM") as dpool:
            idx_sb = pool.tile([128, WT], dt.int16)
            acc = pool.tile([128, SLOTS, C], dt.float32)
            winv_sb = pool.tile([128, SLOTS], dt.float32)
            resp = pool.tile([128, SLOTS], dt.float32)
            wt = pool.tile([128, SLOTS], dt.float32)
            sfac = pool.tile([128, SLOTS], dt.float32)
            red1 = pool.tile([128, 2], dt.float32)
            redc = pool.tile([1, 4], dt.float32)
            thrb = pool.tile([128, 2], dt.float32)
            thresh = pool.tile([128, 1], dt.float32)

            nc.sync.dma_start(out=idx_sb[:], in_=idx[:])
            nc.sync.dma_start(out=winv_sb[:], in_=winv[:])
            nc.vector.memset(acc[:], 0.0)

            # ---- response = sum_c fa^2, in CHUNK-slot chunks ----
            for ci in range(0, SLOTS, CHUNK):
                nch = min(CHUNK, SLOTS - ci)
                fch = fap.tile([128, CHUNK, C], dt.float32, tag="fch")
                sq = fap.tile([128, CHUNK, C], dt.float32, tag="sq")
                nc.sync.dma_start(out=fch[:, :nch, :],
                                  in_=fa[:, ci * C:(ci + nch) * C])
                nc.vector.tensor_mul(sq[:, :nch, :], fch[:, :nch, :], fch[:, :nch, :])
                nc.vector.tensor_reduce(resp[:, ci:ci + nch], sq[:, :nch, :],
                                        mybir.AxisListType.X, mybir.AluOpType.add)

            flat = pool.tile([1, 256], dt.float32)
            packv = pool.tile([1, 2], dt.float32)

            def emit_thresh_block():
                # cross-partition min/max: DMA-flatten [128,2] -> [1,256],
                # then one DVE max-reduce (red1[:,1] is already negated min)
                nc.vector.tensor_reduce(red1[:, 0:1], resp[:],
                                        mybir.AxisListType.X, mybir.AluOpType.max)
                nc.vector.tensor_reduce(red1[:, 1:2], resp[:],
                                        mybir.AxisListType.X, mybir.AluOpType.min)
                nc.vector.tensor_scalar_mul(red1[:, 1:2], red1[:, 1:2], -1.0)
                nc.sync.dma_start(out=flat[:], in_=red1[:])
                nc.vector.tensor_reduce(
                    packv[:], flat[:].rearrange("p (k j) -> p j k", j=2),
                    mybir.AxisListType.X, mybir.AluOpType.max)
                thr2 = pool.tile([1, 2], dt.float32)
                if _knob("NO_COLLECTIVE"):
                    nc.vector.tensor_copy(thr2[:], packv[:])
                else:
                    cc_in = dpool.tile([1, 2], dt.float32)
                    cc_out = dpool.tile([1, 2], dt.float32)
                    nc.sync.dma_start(out=cc_in[:], in_=packv[:])
                    nc.gpsimd.collective_compute(
                        "AllReduce", mybir.AluOpType.max,
                        replica_groups=[list(range(NCORES))],
                        ins=[cc_in.opt()], outs=[cc_out.opt()])
                    nc.sync.dma_start(out=thr2[:], in_=cc_out[:])
                nc.gpsimd.partition_broadcast(thrb[:], thr2[:])
                tmp1 = pool.tile([128, 1], dt.float32)
                nc.vector.tensor_scalar_mul(tmp1[:], thrb[:, 0:1], TAU)
                nc.vector.scalar_tensor_tensor(
                    out=thresh[:], in0=thrb[:, 1:2], scalar=-(1.0 - TAU), in1=tmp1[:],
                    op0=mybir.AluOpType.mult, op1=mybir.AluOpType.add)
                nc.vector.tensor_tensor(wt[:], resp[:],
                                        thresh[:].to_broadcast([128, SLOTS]),
                                        mybir.AluOpType.is_gt)
                nc.vector.tensor_scalar_mul(wt[:], wt[:], ALPHA)
                tmp2 = pool.tile([128, SLOTS], dt.float32)
                nc.vector.tensor_scalar(tmp2[:], wt[:], -1.0, 1.0,
                                        mybir.AluOpType.mult, mybir.AluOpType.add)
                nc.vector.tensor_tensor(sfac[:], tmp2[:], winv_sb[:],
                                        mybir.AluOpType.mult)

            # ---- gather + accumulate pipeline ----
            ninstr_cap = int(os.environ.get("BNNF_MAX_GATHERS", "999999"))
            woff = 0
            for gi, (series, k, s0, ns) in enumerate(plan["instrs"]):
                if gi >= ninstr_cap:
                    break
                nidx = ns * 128
                wcols = ns * 8
                stg = stp.tile([128, 8, C], dt.float32, tag="stage")
                src = ta if series == 1 else tb
                nc.gpsimd.dma_gather(
                    out_ap=stg[:, :ns, :], in_ap=src,
                    idxs_ap=idx_sb[:, woff:woff + wcols],
                    num_idxs=nidx, num_idxs_reg=nidx, elem_size=C)
                aslice = acc[:, s0:s0 + ns, :]
                if series == 1:
                    nc.vector.tensor_add(aslice, aslice, stg[:, :ns, :])
                else:
                    nc.vector.scalar_tensor_tensor(
                        out=aslice, in0=stg[:, :ns, :], scalar=2.0, in1=aslice,
                        op0=mybir.AluOpType.mult, op1=mybir.AluOpType.add)
                woff += wcols
                if gi == 30:
                    emit_thresh_block()

            if ninstr_cap <= 30 or len(plan["instrs"]) <= 30:
                emit_thresh_block()
            # ---- blend: out = fa*w + acc*sfac ----
            for ci in range(0, SLOTS, CHUNK):
                nch = min(CHUNK, SLOTS - ci)
                fch = fap.tile([128, CHUNK, C], dt.float32, tag="fch2")
                nc.sync.dma_start(out=fch[:, :nch, :],
                                  in_=fa[:, ci * C:(ci + nch) * C])
                w_b = wt[:, ci:ci + nch].unsqueeze(2).to_broadcast([128, nch, C])
                s_b = sfac[:, ci:ci + nch].unsqueeze(2).to_broadcast([128, nch, C])
                ach = acc[:, ci:ci + nch, :]
                nc.vector.tensor_tensor(fch[:, :nch, :], fch[:, :nch, :], w_b,
                                        mybir.AluOpType.mult)
                nc.vector.tensor_tensor(ach, ach, s_b, mybir.AluOpType.mult)
                nc.vector.tensor_add(fch[:, :nch, :], fch[:, :nch, :], ach)
                nc.sync.dma_start(out=out[:, ci * C:(ci + nch) * C],
                                  in_=fch[:, :nch, :])
    nc.compile()
    return nc


def _install_ntff_hook():
    try:
        import antenv
        if "antenv.axon_hooks" not in sys.modules:
            mod = types.ModuleType("antenv.axon_hooks")
            _h = [None]
            mod.set_axon_ntff_profile_hook = lambda h: _h.__setitem__(0, h)
            mod.get_axon_ntff_profile_hook = lambda: _h[0]
            sys.modules["antenv.axon_hooks"] = mod
            antenv.axon_hooks = mod
            from trn_agent_boot.trn_boot import _ntff_profile_via_ctypes
            hook = _ntff_profile_via_ctypes('/opt/axon/libaxon_pjrt.so')
            if hook is not None:
                mod.set_axon_ntff_profile_hook(hook)
    except Exception:
        pass


def kernel(ref, f_a, nnf_sr, nnf_rs, _trace=False):
    from concourse.bass_utils import run_bass_kernel_spmd

    _install_ntff_hook()
    ref = np.asarray(ref)
    f_a = np.asarray(f_a)
    plan, in_maps, cores = _prep(ref, nnf_sr, nnf_rs, f_a)

    key = (plan["VA"], plan["VB"], plan["W_total"], tuple(plan["instrs"]),
           os.environ.get("BNNF_NO_COLLECTIVE", ""), os.environ.get("BNNF_MAX_GATHERS", ""))
    if _D.get("key") != key:
        _D["nc"] = _build_program(plan)
        _D["key"] = key
    nc = _D["nc"]

    res = run_bass_kernel_spmd(nc, in_maps, list(range(NCORES)), trace=_trace)
    if _trace:
        _D["exec_time_ns"] = res.exec_time_ns

    outT = np.empty((N, C), np.float32)
    for c, cc in enumerate(cores):
        blob = res.results[c]["out"].reshape(128, SLOTS, C)
        core_rows = blob.transpose(1, 0, 2).reshape(TPC, C)   # rank -> row
        outT[cc["glob"]] = core_rows
    return np.ascontiguousarray(outT.T).reshape(1, C, H, W).astype(np.float32)


# revision 18
# speedup vs baseline: 1.0228x; 1.0228x over previous
"""Trainium2 Bass kernel for bidirectional-NNF patch voting (bds_vote + blend).

Algorithm (mathematically identical to the reference):
  - wr == 2*ws exactly, so guide = (S1 + 2*S2) / (c1 + 2*c2), where S1/c1 are
    the pass-1 (src->ref) vote sum/count and S2/c2 the pass-2 (ref->src) ones.
  - All gather/scatter indexing is channel-independent -> precomputed on host.
  - Both voting passes become per-target gather lists; pass-2 collisions are
    layered (k-th contribution per target).  Each contribution gathers one
    2KB pixel vector (all 512 channels, pixel-major layout).
  - 8 cores each own 4608 target pixels (full channel dim).  Per core the
    distinct gather sources fit in <32K rows, so tables are compacted to make
    indices fit int16 for the GPSIMD dma_gather (InstDMAGatherAnt) primitive.
  - Device: ~90 dma_gather instructions (<=1024 idxs each, descriptor-ring
    limit) -> staging tiles -> DVE accumulate (x1 for pass-1, x2 for pass-2).
    response/min-max via DVE+GPSIMD reduces + a 2-float AllReduce; blend on DVE.
"""
import numpy as np
import os
import sys
import types

sys.path.insert(0, "/opt/trn_rl_repo")


def _knob(name):
    return os.environ.get("BNNF_" + name, "") not in ("", "0")

C, H, W = 512, 192, 192
N = H * W
ALPHA = 0.8
TAU = 0.05
PAD = -1
NCORES = 8
TPC = N // NCORES          # targets per core = 4608
SLOTS = TPC // 128         # acc slots per partition = 36
MAXIDX = 1024              # max idxs per dma_gather (descriptor-ring limit)
CHUNK = 6                  # blend/response chunk: slots per chunk

_D = {}  # module cache for the compiled program


def _build_pass1_planes(nnf_sr):
    ry = nnf_sr[..., 0].astype(np.int64)
    rx = nnf_sr[..., 1].astype(np.int64)
    planes = np.full((9, N), PAD, np.int32)
    k = 0
    for dy in (-1, 0, 1):
        for dx in (-1, 0, 1):
            gy = ry + dy
            gx = rx + dx
            valid = (gy >= 0) & (gy < H) & (gx >= 0) & (gx < W)
            src = np.where(valid, gy * W + gx, PAD).astype(np.int32)
            plane = np.full((H, W), PAD, np.int32)
            ty0, ty1 = max(dy, 0), H + min(dy, 0)
            tx0, tx1 = max(dx, 0), W + min(dx, 0)
            plane[ty0:ty1, tx0:tx1] = src[ty0 - dy:ty1 - dy, tx0 - dx:tx1 - dx]
            planes[k] = plane.ravel()
            k += 1
    return planes


def _build_pass2_planes(nnf_rs):
    sy = nnf_rs[..., 0].astype(np.int64)
    sx = nnf_rs[..., 1].astype(np.int64)
    tgt_all, src_all = [], []
    ryg, rxg = np.meshgrid(np.arange(H), np.arange(W), indexing="ij")
    for dy in (-1, 0, 1):
        for dx in (-1, 0, 1):
            ty = sy + dy
            tx = sx + dx
            gy = ryg + dy
            gx = rxg + dx
            valid = ((ty >= 0) & (ty < H) & (tx >= 0) & (tx < W) &
                     (gy >= 0) & (gy < H) & (gx >= 0) & (gx < W))
            tgt_all.append((ty * W + tx)[valid])
            src_all.append((gy * W + gx)[valid])
    tgt = np.concatenate(tgt_all)
    src = np.concatenate(src_all)
    order = np.argsort(tgt, kind="stable")
    tgt_s, src_s = tgt[order], src[order]
    counts = np.bincount(tgt_s, minlength=N)
    starts = np.concatenate(([0], np.cumsum(counts)[:-1]))
    rank = np.arange(len(tgt_s)) - starts[tgt_s]
    K2 = int(counts.max())
    planes = np.full((K2, N), PAD, np.int32)
    planes[rank, tgt_s] = src_s
    return planes, counts


def _wrap_idx(ix):
    """[n] int -> [128, n//16] int16 (wrapped in 16 partitions, replicated x8)."""
    return np.tile(ix.astype(np.int16).reshape(-1, 16).T, (8, 1))


def _prep(ref, nnf_sr, nnf_rs, f_a):
    """Host preprocessing. Returns (plan, per-core input maps skeleton)."""
    idxA = _build_pass1_planes(np.asarray(nnf_sr))
    idxB, c2 = _build_pass2_planes(np.asarray(nnf_rs))
    K2 = idxB.shape[0]

    refT = np.ascontiguousarray(np.asarray(ref).reshape(C, N).T)     # [N, C]
    faT = np.asarray(f_a).reshape(C, N).T                            # [N, C] (view)

    gorder = np.argsort(-c2, kind="stable")         # global rank -> target
    cores = []
    for c in range(NCORES):
        glob = gorder[c::NCORES]                    # local rank -> global target
        c2c = c2[glob]                              # descending by construction
        n_k = np.array([(c2c > k).sum() for k in range(K2)], np.int64)

        srcA = idxA[:, glob]                        # [9, TPC] global rows or PAD
        srcB = idxB[:, glob]                        # [K2, TPC]

        va = srcA[srcA != PAD]
        uniqA, invA = np.unique(va, return_inverse=True)
        remA = np.full(srcA.shape, PAD, np.int32)
        remA[srcA != PAD] = invA.astype(np.int32)

        vb = srcB[srcB != PAD]
        uniqB, invB = np.unique(vb, return_inverse=True)
        remB = np.full(srcB.shape, PAD, np.int32)
        remB[srcB != PAD] = invB.astype(np.int32)
        # pads point at a zeros row appended to each table (index = n_uniq):
        # gathers write zeros -> accumulate is a no-op, and we avoid the
        # GatherAnt negative-index tail path which wedges the device.
        remA[remA == PAD] = len(uniqA)
        remB[remB == PAD] = len(uniqB)

        cores.append(dict(glob=glob, n_k=n_k, remA=remA, remB=remB,
                          uniqA=uniqA, uniqB=uniqB))

    VA = max(len(cc["uniqA"]) for cc in cores) + 1
    VB = max(len(cc["uniqB"]) for cc in cores) + 1
    assert VA <= 32767 and VB <= 32767, (VA, VB)
    n_k_max = np.max([cc["n_k"] for cc in cores], axis=0)            # [K2]

    # instruction plan shared by all cores: (series, slot_off, nslots)
    instrs = []
    for k in range(9):                      # series 1: full slabs
        for s0 in range(0, SLOTS, 8):
            ns = min(8, SLOTS - s0)
            instrs.append((1, k, s0, ns))
    for k in range(K2):                     # series 2: prefix slabs
        sk = int(np.ceil(n_k_max[k] / 128))
        if sk == 0:
            continue
        for s0 in range(0, sk, 8):
            ns = min(8, sk - s0)
            instrs.append((2, k, s0, ns))

    # per-core idx blobs + tables
    in_maps = []
    for cc in cores:
        blobs = []
        for (series, k, s0, ns) in instrs:
            rem = cc["remA"] if series == 1 else cc["remB"]
            zrow = len(cc["uniqA"]) if series == 1 else len(cc["uniqB"])
            rr = np.arange(s0 * 128, (s0 + ns) * 128)
            ix = rem[k][rr] if k < rem.shape[0] else np.full(ns * 128, zrow, np.int32)
            blobs.append(_wrap_idx(ix))
        idx_blob = np.ascontiguousarray(np.concatenate(blobs, axis=1))

        TA = np.zeros((VA, C), np.float32)
        TA[:len(cc["uniqA"])] = refT[cc["uniqA"]]
        TB = np.zeros((VB, C), np.float32)
        TB[:len(cc["uniqB"])] = refT[cc["uniqB"]]

        fa_core = faT[cc["glob"]]                                    # [TPC, C]
        fa_blob = np.ascontiguousarray(
            fa_core.reshape(SLOTS, 128, C).transpose(1, 0, 2)).reshape(128, SLOTS * C)

        c1 = (cc["remA"] != len(cc["uniqA"])).sum(axis=0)
        c2r = (cc["remB"] != len(cc["uniqB"])).sum(axis=0)
        den = (c1 + 2 * c2r).astype(np.float32)
        winv = np.where(den == 0, np.float32(1.0), 1.0 / np.maximum(den, 1)).astype(np.float32)
        winv_blob = np.ascontiguousarray(winv.reshape(SLOTS, 128).T)  # [128, SLOTS]

        in_maps.append({"ta": TA, "tb": TB, "idx": idx_blob.astype(np.int16),
                        "fa": fa_blob.astype(np.float32), "winv": winv_blob})

    plan = dict(instrs=instrs, VA=VA, VB=VB,
                W_total=sum(ns * 8 for (_, _, _, ns) in instrs))
    return plan, in_maps, cores


def _build_program(plan):
    from concourse import bacc, bass, mybir, tile

    VA, VB = plan["VA"], plan["VB"]
    WT = plan["W_total"]
    nc = bacc.Bacc("TRN2", target_bir_lowering=False, debug=False,
                   num_devices=NCORES)
    dt = mybir.dt
    ta = nc.dram_tensor("ta", [VA, C], dt.float32, kind="ExternalInput").ap()
    tb = nc.dram_tensor("tb", [VB, C], dt.float32, kind="ExternalInput").ap()
    idx = nc.dram_tensor("idx", [128, WT], dt.int16, kind="ExternalInput").ap()
    fa = nc.dram_tensor("fa", [128, SLOTS * C], dt.float32, kind="ExternalInput").ap()
    winv = nc.dram_tensor("winv", [128, SLOTS], dt.float32, kind="ExternalInput").ap()
    out = nc.dram_tensor("out", [128, SLOTS * C], dt.float32, kind="ExternalOutput").ap()

    with tile.TileContext(nc) as tc:
        with tc.tile_pool(name="sbuf", bufs=1) as pool, \
             tc.tile_pool(name="stg", bufs=3) as stp, \
             tc.tile_pool(name="fac", bufs=2) as fap, \
             tc.tile_pool(name="dram", bufs=1, space="DRAM") as dpool:
            idx_sb = pool.tile([128, WT], dt.int16)
            acc = pool.tile([128, SLOTS, C], dt.float32)
            winv_sb = pool.tile([128, SLOTS], dt.float32)
            resp = pool.tile([128, SLOTS], dt.float32)
            wt = pool.tile([128, SLOTS], dt.float32)
            sfac = pool.tile([128, SLOTS], dt.float32)
            red1 = pool.tile([128, 2], dt.float32)
            redc = pool.tile([1, 4], dt.float32)
            thrb = pool.tile([128, 2], dt.float32)
            thresh = pool.tile([128, 1], dt.float32)

            nc.sync.dma_start(out=idx_sb[:], in_=idx[:])
            nc.sync.dma_start(out=winv_sb[:], in_=winv[:])
            nc.vector.memset(acc[:], 0.0)

            # ---- response = sum_c fa^2, in CHUNK-slot chunks ----
            for ci in range(0, SLOTS, CHUNK):
                nch = min(CHUNK, SLOTS - ci)
                fch = fap.tile([128, CHUNK, C], dt.float32, tag="fch")
                sq = fap.tile([128, CHUNK, C], dt.float32, tag="sq")
                nc.sync.dma_start(out=fch[:, :nch, :],
                                  in_=fa[:, ci * C:(ci + nch) * C])
                nc.vector.tensor_mul(sq[:, :nch, :], fch[:, :nch, :], fch[:, :nch, :])
                nc.vector.tensor_reduce(resp[:, ci:ci + nch], sq[:, :nch, :],
                                        mybir.AxisListType.X, mybir.AluOpType.add)

            flat = pool.tile([1, 256], dt.float32)
            packv = pool.tile([1, 2], dt.float32)

            def emit_thresh_block():
                # cross-partition min/max: DMA-flatten [128,2] -> [1,256],
                # then one DVE max-reduce (red1[:,1] is already negated min)
                nc.vector.tensor_reduce(red1[:, 0:1], resp[:],
                                        mybir.AxisListType.X, mybir.AluOpType.max)
                nc.vector.tensor_reduce(red1[:, 1:2], resp[:],
                                        mybir.AxisListType.X, mybir.AluOpType.min)
                nc.vector.tensor_scalar_mul(red1[:, 1:2], red1[:, 1:2], -1.0)
                nc.sync.dma_start(out=flat[:], in_=red1[:])
                nc.vector.tensor_reduce(
                    packv[:], flat[:].rearrange("p (k j) -> p j k", j=2),
                    mybir.AxisListType.X, mybir.AluOpType.max)
                thr2 = pool.tile([1, 2], dt.float32)
                if _knob("NO_COLLECTIVE"):
                    nc.vector.tensor_copy(thr2[:], packv[:])
                else:
                    cc_in = dpool.tile([1, 2], dt.float32)
                    cc_out = dpool.tile([1, 2], dt.float32)
                    nc.sync.dma_start(out=cc_in[:], in_=packv[:])
                    nc.gpsimd.collective_compute(
                        "AllReduce", mybir.AluOpType.max,
                        replica_groups=[list(range(NCORES))],
                        ins=[cc_in.opt()], outs=[cc_out.opt()])
                    nc.sync.dma_start(out=thr2[:], in_=cc_out[:])
                nc.gpsimd.partition_broadcast(thrb[:], thr2[:])
                tmp1 = pool.tile([128, 1], dt.float32)
                nc.vector.tensor_scalar_mul(tmp1[:], thrb[:, 0:1], TAU)
                nc.vector.scalar_tensor_tensor(
                    out=thresh[:], in0=thrb[:, 1:2], scalar=-(1.0 - TAU), in1=tmp1[:],
                    op0=mybir.AluOpType.mult, op1=mybir.AluOpType.add)
                nc.vector.tensor_tensor(wt[:], resp[:],
                                        thresh[:].to_broadcast([128, SLOTS]),
                                        mybir.AluOpType.is_gt)
                nc.vector.tensor_scalar_mul(wt[:], wt[:], ALPHA)
                tmp2 = pool.tile([128, SLOTS], dt.float32)
                nc.vector.tensor_scalar(tmp2[:], wt[:], -1.0, 1.0,
                                        mybir.AluOpType.mult, mybir.AluOpType.add)
                nc.vector.tensor_tensor(sfac[:], tmp2[:], winv_sb[:],
                                        mybir.AluOpType.mult)

            # ---- gather + accumulate pipeline ----
            ninstr_cap = int(os.environ.get("BNNF_MAX_GATHERS", "999999"))
            woff = 0
            for gi, (series, k, s0, ns) in enumerate(plan["instrs"]):
                if gi >= ninstr_cap:
                    break
                nidx = ns * 128
                wcols = ns * 8
                stg = stp.tile([128, 8, C], dt.float32, tag="stage")
                src = ta if series == 1 else tb
                nc.gpsimd.dma_gather(
                    out_ap=stg[:, :ns, :], in_ap=src,
                    idxs_ap=idx_sb[:, woff:woff + wcols],
                    num_idxs=nidx, num_idxs_reg=nidx, elem_size=C, single_packet=False)
                aslice = acc[:, s0:s0 + ns, :]
                if series == 1:
                    nc.vector.tensor_add(aslice, aslice, stg[:, :ns, :])
                else:
                    nc.vector.scalar_tensor_tensor(
                        out=aslice, in0=stg[:, :ns, :], scalar=2.0, in1=aslice,
                        op0=mybir.AluOpType.mult, op1=mybir.AluOpType.add)
                woff += wcols
                if gi == 30:
                    emit_thresh_block()

            if ninstr_cap <= 30 or len(plan["instrs"]) <= 30:
                emit_thresh_block()
            # ---- blend: out = fa*w + acc*sfac ----
            for ci in range(0, SLOTS, CHUNK):
                nch = min(CHUNK, SLOTS - ci)
                fch = fap.tile([128, CHUNK, C], dt.float32, tag="fch2")
                nc.sync.dma_start(out=fch[:, :nch, :],
                                  in_=fa[:, ci * C:(ci + nch) * C])
                w_b = wt[:, ci:ci + nch].unsqueeze(2).to_broadcast([128, nch, C])
                s_b = sfac[:, ci:ci + nch].unsqueeze(2).to_broadcast([128, nch, C])
                ach = acc[:, ci:ci + nch, :]
                nc.vector.tensor_tensor(fch[:, :nch, :], fch[:, :nch, :], w_b,
                                        mybir.AluOpType.mult)
                nc.vector.tensor_tensor(ach, ach, s_b, mybir.AluOpType.mult)
                nc.vector.tensor_add(fch[:, :nch, :], fch[:, :nch, :], ach)
                nc.sync.dma_start(out=out[:, ci * C:(ci + nch) * C],
                                  in_=fch[:, :nch, :])
    nc.compile()
    return nc


def _install_ntff_hook():
    try:
        import antenv
        if "antenv.axon_hooks" not in sys.modules:
            mod = types.ModuleType("antenv.axon_hooks")
            _h = [None]
            mod.set_axon_ntff_profile_hook = lambda h: _h.__setitem__(0, h)
            mod.get_axon_ntff_profile_hook = lambda: _h[0]
            sys.modules["antenv.axon_hooks"] = mod
            antenv.axon_hooks = mod
            from trn_agent_boot.trn_boot import _ntff_profile_via_ctypes
            hook = _ntff_profile_via_ctypes('/opt/axon/libaxon_pjrt.so')
            if hook is not None:
                mod.set_axon_ntff_profile_hook(hook)
    except Exception:
        pass


def kernel(ref, f_a, nnf_sr, nnf_rs, _trace=False):
    from concourse.bass_utils import run_bass_kernel_spmd

    _install_ntff_hook()
    ref = np.asarray(ref)
    f_a = np.asarray(f_a)
    plan, in_maps, cores = _prep(ref, nnf_sr, nnf_rs, f_a)

    key = (plan["VA"], plan["VB"], plan["W_total"], tuple(plan["instrs"]),
           os.environ.get("BNNF_NO_COLLECTIVE", ""), os.environ.get("BNNF_MAX_GATHERS", ""))
    if _D.get("key") != key:
        _D["nc"] = _build_program(plan)
        _D["key"] = key
    nc = _D["nc"]

    res = run_bass_kernel_spmd(nc, in_maps, list(range(NCORES)), trace=_trace)
    if _trace:
        _D["exec_time_ns"] = res.exec_time_ns

    outT = np.empty((N, C), np.float32)
    for c, cc in enumerate(cores):
        blob = res.results[c]["out"].reshape(128, SLOTS, C)
        core_rows = blob.transpose(1, 0, 2).reshape(TPC, C)   # rank -> row
        outT[cc["glob"]] = core_rows
    return np.ascontiguousarray(outT.T).reshape(1, C, H, W).astype(np.float32)


# revision 19
# speedup vs baseline: 1.0257x; 1.0028x over previous
"""Trainium2 Bass kernel for bidirectional-NNF patch voting (bds_vote + blend).

Algorithm (mathematically identical to the reference):
  - wr == 2*ws exactly, so guide = (S1 + 2*S2) / (c1 + 2*c2), where S1/c1 are
    the pass-1 (src->ref) vote sum/count and S2/c2 the pass-2 (ref->src) ones.
  - All gather/scatter indexing is channel-independent -> precomputed on host.
  - Both voting passes become per-target gather lists; pass-2 collisions are
    layered (k-th contribution per target).  Each contribution gathers one
    2KB pixel vector (all 512 channels, pixel-major layout).
  - 8 cores each own 4608 target pixels (full channel dim).  Per core the
    distinct gather sources fit in <32K rows, so tables are compacted to make
    indices fit int16 for the GPSIMD dma_gather (InstDMAGatherAnt) primitive.
  - Device: ~90 dma_gather instructions (<=1024 idxs each, descriptor-ring
    limit) -> staging tiles -> DVE accumulate (x1 for pass-1, x2 for pass-2).
    response/min-max via DVE+GPSIMD reduces + a 2-float AllReduce; blend on DVE.
"""
import numpy as np
import os
import sys
import types

sys.path.insert(0, "/opt/trn_rl_repo")


def _knob(name):
    return os.environ.get("BNNF_" + name, "") not in ("", "0")

C, H, W = 512, 192, 192
N = H * W
ALPHA = 0.8
TAU = 0.05
PAD = -1
NCORES = 8
TPC = N // NCORES          # targets per core = 4608
SLOTS = TPC // 128         # acc slots per partition = 36
MAXIDX = 1024              # max idxs per dma_gather (descriptor-ring limit)
CHUNK = 6                  # blend/response chunk: slots per chunk

_D = {}  # module cache for the compiled program


def _build_pass1_planes(nnf_sr):
    ry = nnf_sr[..., 0].astype(np.int64)
    rx = nnf_sr[..., 1].astype(np.int64)
    planes = np.full((9, N), PAD, np.int32)
    k = 0
    for dy in (-1, 0, 1):
        for dx in (-1, 0, 1):
            gy = ry + dy
            gx = rx + dx
            valid = (gy >= 0) & (gy < H) & (gx >= 0) & (gx < W)
            src = np.where(valid, gy * W + gx, PAD).astype(np.int32)
            plane = np.full((H, W), PAD, np.int32)
            ty0, ty1 = max(dy, 0), H + min(dy, 0)
            tx0, tx1 = max(dx, 0), W + min(dx, 0)
            plane[ty0:ty1, tx0:tx1] = src[ty0 - dy:ty1 - dy, tx0 - dx:tx1 - dx]
            planes[k] = plane.ravel()
            k += 1
    return planes


def _build_pass2_planes(nnf_rs):
    sy = nnf_rs[..., 0].astype(np.int64)
    sx = nnf_rs[..., 1].astype(np.int64)
    tgt_all, src_all = [], []
    ryg, rxg = np.meshgrid(np.arange(H), np.arange(W), indexing="ij")
    for dy in (-1, 0, 1):
        for dx in (-1, 0, 1):
            ty = sy + dy
            tx = sx + dx
            gy = ryg + dy
            gx = rxg + dx
            valid = ((ty >= 0) & (ty < H) & (tx >= 0) & (tx < W) &
                     (gy >= 0) & (gy < H) & (gx >= 0) & (gx < W))
            tgt_all.append((ty * W + tx)[valid])
            src_all.append((gy * W + gx)[valid])
    tgt = np.concatenate(tgt_all)
    src = np.concatenate(src_all)
    order = np.argsort(tgt, kind="stable")
    tgt_s, src_s = tgt[order], src[order]
    counts = np.bincount(tgt_s, minlength=N)
    starts = np.concatenate(([0], np.cumsum(counts)[:-1]))
    rank = np.arange(len(tgt_s)) - starts[tgt_s]
    K2 = int(counts.max())
    planes = np.full((K2, N), PAD, np.int32)
    planes[rank, tgt_s] = src_s
    return planes, counts


def _wrap_idx(ix):
    """[n] int -> [128, n//16] int16 (wrapped in 16 partitions, replicated x8)."""
    return np.tile(ix.astype(np.int16).reshape(-1, 16).T, (8, 1))


def _prep(ref, nnf_sr, nnf_rs, f_a):
    """Host preprocessing. Returns (plan, per-core input maps skeleton)."""
    idxA = _build_pass1_planes(np.asarray(nnf_sr))
    idxB, c2 = _build_pass2_planes(np.asarray(nnf_rs))
    K2 = idxB.shape[0]

    refT = np.ascontiguousarray(np.asarray(ref).reshape(C, N).T)     # [N, C]
    faT = np.asarray(f_a).reshape(C, N).T                            # [N, C] (view)

    gorder = np.argsort(-c2, kind="stable")         # global rank -> target
    cores = []
    for c in range(NCORES):
        glob = gorder[c::NCORES]                    # local rank -> global target
        c2c = c2[glob]                              # descending by construction
        n_k = np.array([(c2c > k).sum() for k in range(K2)], np.int64)

        srcA = idxA[:, glob]                        # [9, TPC] global rows or PAD
        srcB = idxB[:, glob]                        # [K2, TPC]

        def compact_first_use(srcM):
            # order table rows by first use in the (k-major) gather stream so
            # consecutive descriptors hit nearby HBM rows
            stream = srcM.ravel()
            valid = stream != PAD
            vals, firsts = np.unique(stream[valid], return_index=True)
            uniq = vals[np.argsort(firsts)]
            lut = np.full(N, PAD, np.int32)
            lut[uniq] = np.arange(len(uniq), dtype=np.int32)
            rem = np.full(srcM.shape, PAD, np.int32)
            rem[srcM != PAD] = lut[srcM[srcM != PAD]]
            return uniq, rem

        uniqA, remA = compact_first_use(srcA)
        uniqB, remB = compact_first_use(srcB)
        # pads point at a zeros row appended to each table (index = n_uniq):
        # gathers write zeros -> accumulate is a no-op, and we avoid the
        # GatherAnt negative-index tail path which wedges the device.
        remA[remA == PAD] = len(uniqA)
        remB[remB == PAD] = len(uniqB)

        cores.append(dict(glob=glob, n_k=n_k, remA=remA, remB=remB,
                          uniqA=uniqA, uniqB=uniqB))

    VA = max(len(cc["uniqA"]) for cc in cores) + 1
    VB = max(len(cc["uniqB"]) for cc in cores) + 1
    assert VA <= 32767 and VB <= 32767, (VA, VB)
    n_k_max = np.max([cc["n_k"] for cc in cores], axis=0)            # [K2]

    # instruction plan shared by all cores: (series, slot_off, nslots)
    instrs = []
    for k in range(9):                      # series 1: full slabs
        for s0 in range(0, SLOTS, 8):
            ns = min(8, SLOTS - s0)
            instrs.append((1, k, s0, ns))
    for k in range(K2):                     # series 2: prefix slabs
        sk = int(np.ceil(n_k_max[k] / 128))
        if sk == 0:
            continue
        for s0 in range(0, sk, 8):
            ns = min(8, sk - s0)
            instrs.append((2, k, s0, ns))

    # per-core idx blobs + tables
    in_maps = []
    for cc in cores:
        blobs = []
        for (series, k, s0, ns) in instrs:
            rem = cc["remA"] if series == 1 else cc["remB"]
            zrow = len(cc["uniqA"]) if series == 1 else len(cc["uniqB"])
            rr = np.arange(s0 * 128, (s0 + ns) * 128)
            ix = rem[k][rr] if k < rem.shape[0] else np.full(ns * 128, zrow, np.int32)
            blobs.append(_wrap_idx(ix))
        idx_blob = np.ascontiguousarray(np.concatenate(blobs, axis=1))

        TA = np.zeros((VA, C), np.float32)
        TA[:len(cc["uniqA"])] = refT[cc["uniqA"]]
        TB = np.zeros((VB, C), np.float32)
        TB[:len(cc["uniqB"])] = refT[cc["uniqB"]]

        fa_core = faT[cc["glob"]]                                    # [TPC, C]
        fa_blob = np.ascontiguousarray(
            fa_core.reshape(SLOTS, 128, C).transpose(1, 0, 2)).reshape(128, SLOTS * C)

        c1 = (cc["remA"] != len(cc["uniqA"])).sum(axis=0)
        c2r = (cc["remB"] != len(cc["uniqB"])).sum(axis=0)
        den = (c1 + 2 * c2r).astype(np.float32)
        winv = np.where(den == 0, np.float32(1.0), 1.0 / np.maximum(den, 1)).astype(np.float32)
        winv_blob = np.ascontiguousarray(winv.reshape(SLOTS, 128).T)  # [128, SLOTS]

        in_maps.append({"ta": TA, "tb": TB, "idx": idx_blob.astype(np.int16),
                        "fa": fa_blob.astype(np.float32), "winv": winv_blob})

    plan = dict(instrs=instrs, VA=VA, VB=VB,
                W_total=sum(ns * 8 for (_, _, _, ns) in instrs))
    return plan, in_maps, cores


def _build_program(plan):
    from concourse import bacc, bass, mybir, tile

    VA, VB = plan["VA"], plan["VB"]
    WT = plan["W_total"]
    nc = bacc.Bacc("TRN2", target_bir_lowering=False, debug=False,
                   num_devices=NCORES)
    dt = mybir.dt
    ta = nc.dram_tensor("ta", [VA, C], dt.float32, kind="ExternalInput").ap()
    tb = nc.dram_tensor("tb", [VB, C], dt.float32, kind="ExternalInput").ap()
    idx = nc.dram_tensor("idx", [128, WT], dt.int16, kind="ExternalInput").ap()
    fa = nc.dram_tensor("fa", [128, SLOTS * C], dt.float32, kind="ExternalInput").ap()
    winv = nc.dram_tensor("winv", [128, SLOTS], dt.float32, kind="ExternalInput").ap()
    out = nc.dram_tensor("out", [128, SLOTS * C], dt.float32, kind="ExternalOutput").ap()

    with tile.TileContext(nc) as tc:
        with tc.tile_pool(name="sbuf", bufs=1) as pool, \
             tc.tile_pool(name="stg", bufs=3) as stp, \
             tc.tile_pool(name="fac", bufs=2) as fap, \
             tc.tile_pool(name="dram", bufs=1, space="DRAM") as dpool:
            idx_sb = pool.tile([128, WT], dt.int16)
            acc = pool.tile([128, SLOTS, C], dt.float32)
            winv_sb = pool.tile([128, SLOTS], dt.float32)
            resp = pool.tile([128, SLOTS], dt.float32)
            wt = pool.tile([128, SLOTS], dt.float32)
            sfac = pool.tile([128, SLOTS], dt.float32)
            red1 = pool.tile([128, 2], dt.float32)
            redc = pool.tile([1, 4], dt.float32)
            thrb = pool.tile([128, 2], dt.float32)
            thresh = pool.tile([128, 1], dt.float32)

            nc.sync.dma_start(out=idx_sb[:], in_=idx[:])
            nc.sync.dma_start(out=winv_sb[:], in_=winv[:])
            nc.vector.memset(acc[:], 0.0)

            # ---- response = sum_c fa^2, in CHUNK-slot chunks ----
            for ci in range(0, SLOTS, CHUNK):
                nch = min(CHUNK, SLOTS - ci)
                fch = fap.tile([128, CHUNK, C], dt.float32, tag="fch")
                sq = fap.tile([128, CHUNK, C], dt.float32, tag="sq")
                nc.sync.dma_start(out=fch[:, :nch, :],
                                  in_=fa[:, ci * C:(ci + nch) * C])
                nc.vector.tensor_mul(sq[:, :nch, :], fch[:, :nch, :], fch[:, :nch, :])
                nc.vector.tensor_reduce(resp[:, ci:ci + nch], sq[:, :nch, :],
                                        mybir.AxisListType.X, mybir.AluOpType.add)

            flat = pool.tile([1, 256], dt.float32)
            packv = pool.tile([1, 2], dt.float32)

            def emit_thresh_block():
                # cross-partition min/max: DMA-flatten [128,2] -> [1,256],
                # then one DVE max-reduce (red1[:,1] is already negated min)
                nc.vector.tensor_reduce(red1[:, 0:1], resp[:],
                                        mybir.AxisListType.X, mybir.AluOpType.max)
                nc.vector.tensor_reduce(red1[:, 1:2], resp[:],
                                        mybir.AxisListType.X, mybir.AluOpType.min)
                nc.vector.tensor_scalar_mul(red1[:, 1:2], red1[:, 1:2], -1.0)
                nc.sync.dma_start(out=flat[:], in_=red1[:])
                nc.vector.tensor_reduce(
                    packv[:], flat[:].rearrange("p (k j) -> p j k", j=2),
                    mybir.AxisListType.X, mybir.AluOpType.max)
                thr2 = pool.tile([1, 2], dt.float32)
                if _knob("NO_COLLECTIVE"):
                    nc.vector.tensor_copy(thr2[:], packv[:])
                else:
                    cc_in = dpool.tile([1, 2], dt.float32)
                    cc_out = dpool.tile([1, 2], dt.float32)
                    nc.sync.dma_start(out=cc_in[:], in_=packv[:])
                    nc.gpsimd.collective_compute(
                        "AllReduce", mybir.AluOpType.max,
                        replica_groups=[list(range(NCORES))],
                        ins=[cc_in.opt()], outs=[cc_out.opt()])
                    nc.sync.dma_start(out=thr2[:], in_=cc_out[:])
                nc.gpsimd.partition_broadcast(thrb[:], thr2[:])
                tmp1 = pool.tile([128, 1], dt.float32)
                nc.vector.tensor_scalar_mul(tmp1[:], thrb[:, 0:1], TAU)
                nc.vector.scalar_tensor_tensor(
                    out=thresh[:], in0=thrb[:, 1:2], scalar=-(1.0 - TAU), in1=tmp1[:],
                    op0=mybir.AluOpType.mult, op1=mybir.AluOpType.add)
                nc.vector.tensor_tensor(wt[:], resp[:],
                                        thresh[:].to_broadcast([128, SLOTS]),
                                        mybir.AluOpType.is_gt)
                nc.vector.tensor_scalar_mul(wt[:], wt[:], ALPHA)
                tmp2 = pool.tile([128, SLOTS], dt.float32)
                nc.vector.tensor_scalar(tmp2[:], wt[:], -1.0, 1.0,
                                        mybir.AluOpType.mult, mybir.AluOpType.add)
                nc.vector.tensor_tensor(sfac[:], tmp2[:], winv_sb[:],
                                        mybir.AluOpType.mult)

            # ---- gather + accumulate pipeline ----
            ninstr_cap = int(os.environ.get("BNNF_MAX_GATHERS", "999999"))
            woff = 0
            for gi, (series, k, s0, ns) in enumerate(plan["instrs"]):
                if gi >= ninstr_cap:
                    break
                nidx = ns * 128
                wcols = ns * 8
                stg = stp.tile([128, 8, C], dt.float32, tag="stage")
                src = ta if series == 1 else tb
                nc.gpsimd.dma_gather(
                    out_ap=stg[:, :ns, :], in_ap=src,
                    idxs_ap=idx_sb[:, woff:woff + wcols],
                    num_idxs=nidx, num_idxs_reg=nidx, elem_size=C, single_packet=False)
                aslice = acc[:, s0:s0 + ns, :]
                if series == 1:
                    nc.vector.tensor_add(aslice, aslice, stg[:, :ns, :])
                else:
                    nc.vector.scalar_tensor_tensor(
                        out=aslice, in0=stg[:, :ns, :], scalar=2.0, in1=aslice,
                        op0=mybir.AluOpType.mult, op1=mybir.AluOpType.add)
                woff += wcols
                if gi == 30:
                    emit_thresh_block()

            if ninstr_cap <= 30 or len(plan["instrs"]) <= 30:
                emit_thresh_block()
            # ---- blend: out = fa*w + acc*sfac ----
            for ci in range(0, SLOTS, CHUNK):
                nch = min(CHUNK, SLOTS - ci)
                fch = fap.tile([128, CHUNK, C], dt.float32, tag="fch2")
                nc.sync.dma_start(out=fch[:, :nch, :],
                                  in_=fa[:, ci * C:(ci + nch) * C])
                w_b = wt[:, ci:ci + nch].unsqueeze(2).to_broadcast([128, nch, C])
                s_b = sfac[:, ci:ci + nch].unsqueeze(2).to_broadcast([128, nch, C])
                ach = acc[:, ci:ci + nch, :]
                nc.vector.tensor_tensor(fch[:, :nch, :], fch[:, :nch, :], w_b,
                                        mybir.AluOpType.mult)
                nc.vector.tensor_tensor(ach, ach, s_b, mybir.AluOpType.mult)
                nc.vector.tensor_add(fch[:, :nch, :], fch[:, :nch, :], ach)
                nc.sync.dma_start(out=out[:, ci * C:(ci + nch) * C],
                                  in_=fch[:, :nch, :])
    nc.compile()
    return nc


def _install_ntff_hook():
    try:
        import antenv
        if "antenv.axon_hooks" not in sys.modules:
            mod = types.ModuleType("antenv.axon_hooks")
            _h = [None]
            mod.set_axon_ntff_profile_hook = lambda h: _h.__setitem__(0, h)
            mod.get_axon_ntff_profile_hook = lambda: _h[0]
            sys.modules["antenv.axon_hooks"] = mod
            antenv.axon_hooks = mod
            from trn_agent_boot.trn_boot import _ntff_profile_via_ctypes
            hook = _ntff_profile_via_ctypes('/opt/axon/libaxon_pjrt.so')
            if hook is not None:
                mod.set_axon_ntff_profile_hook(hook)
    except Exception:
        pass


def kernel(ref, f_a, nnf_sr, nnf_rs, _trace=False):
    from concourse.bass_utils import run_bass_kernel_spmd

    _install_ntff_hook()
    ref = np.asarray(ref)
    f_a = np.asarray(f_a)
    plan, in_maps, cores = _prep(ref, nnf_sr, nnf_rs, f_a)

    key = (plan["VA"], plan["VB"], plan["W_total"], tuple(plan["instrs"]),
           os.environ.get("BNNF_NO_COLLECTIVE", ""), os.environ.get("BNNF_MAX_GATHERS", ""))
    if _D.get("key") != key:
        _D["nc"] = _build_program(plan)
        _D["key"] = key
    nc = _D["nc"]

    res = run_bass_kernel_spmd(nc, in_maps, list(range(NCORES)), trace=_trace)
    if _trace:
        _D["exec_time_ns"] = res.exec_time_ns

    outT = np.empty((N, C), np.float32)
    for c, cc in enumerate(cores):
        blob = res.results[c]["out"].reshape(128, SLOTS, C)
        core_rows = blob.transpose(1, 0, 2).reshape(TPC, C)   # rank -> row
        outT[cc["glob"]] = core_rows
    return np.ascontiguousarray(outT.T).reshape(1, C, H, W).astype(np.float32)


# revision 20
# speedup vs baseline: 1.0310x; 1.0052x over previous
"""Trainium2 Bass kernel for bidirectional-NNF patch voting (bds_vote + blend).

Algorithm (mathematically identical to the reference):
  - wr == 2*ws exactly, so guide = (S1 + 2*S2) / (c1 + 2*c2), where S1/c1 are
    the pass-1 (src->ref) vote sum/count and S2/c2 the pass-2 (ref->src) ones.
  - All gather/scatter indexing is channel-independent -> precomputed on host.
  - Both voting passes become per-target gather lists; pass-2 collisions are
    layered (k-th contribution per target).  Each contribution gathers one
    2KB pixel vector (all 512 channels, pixel-major layout).
  - 8 cores each own 4608 target pixels (full channel dim).  Per core the
    distinct gather sources fit in <32K rows, so tables are compacted to make
    indices fit int16 for the GPSIMD dma_gather (InstDMAGatherAnt) primitive.
  - Device: ~90 dma_gather instructions (<=1024 idxs each, descriptor-ring
    limit) -> staging tiles -> DVE accumulate (x1 for pass-1, x2 for pass-2).
    response/min-max via DVE+GPSIMD reduces + a 2-float AllReduce; blend on DVE.
"""
import numpy as np
import os
import sys
import types

sys.path.insert(0, "/opt/trn_rl_repo")


def _knob(name):
    return os.environ.get("BNNF_" + name, "") not in ("", "0")

C, H, W = 512, 192, 192
N = H * W
ALPHA = 0.8
TAU = 0.05
PAD = -1
NCORES = 8
TPC = N // NCORES          # targets per core = 4608
SLOTS = TPC // 128         # acc slots per partition = 36
MAXIDX = 1024              # max idxs per dma_gather (descriptor-ring limit)
CHUNK = 6                  # blend/response chunk: slots per chunk

_D = {}  # module cache for the compiled program


def _build_pass1_planes(nnf_sr):
    ry = nnf_sr[..., 0].astype(np.int64)
    rx = nnf_sr[..., 1].astype(np.int64)
    planes = np.full((9, N), PAD, np.int32)
    k = 0
    for dy in (-1, 0, 1):
        for dx in (-1, 0, 1):
            gy = ry + dy
            gx = rx + dx
            valid = (gy >= 0) & (gy < H) & (gx >= 0) & (gx < W)
            src = np.where(valid, gy * W + gx, PAD).astype(np.int32)
            plane = np.full((H, W), PAD, np.int32)
            ty0, ty1 = max(dy, 0), H + min(dy, 0)
            tx0, tx1 = max(dx, 0), W + min(dx, 0)
            plane[ty0:ty1, tx0:tx1] = src[ty0 - dy:ty1 - dy, tx0 - dx:tx1 - dx]
            planes[k] = plane.ravel()
            k += 1
    return planes


def _build_pass2_planes(nnf_rs):
    sy = nnf_rs[..., 0].astype(np.int64)
    sx = nnf_rs[..., 1].astype(np.int64)
    tgt_all, src_all = [], []
    ryg, rxg = np.meshgrid(np.arange(H), np.arange(W), indexing="ij")
    for dy in (-1, 0, 1):
        for dx in (-1, 0, 1):
            ty = sy + dy
            tx = sx + dx
            gy = ryg + dy
            gx = rxg + dx
            valid = ((ty >= 0) & (ty < H) & (tx >= 0) & (tx < W) &
                     (gy >= 0) & (gy < H) & (gx >= 0) & (gx < W))
            tgt_all.append((ty * W + tx)[valid])
            src_all.append((gy * W + gx)[valid])
    tgt = np.concatenate(tgt_all)
    src = np.concatenate(src_all)
    order = np.argsort(tgt, kind="stable")
    tgt_s, src_s = tgt[order], src[order]
    counts = np.bincount(tgt_s, minlength=N)
    starts = np.concatenate(([0], np.cumsum(counts)[:-1]))
    rank = np.arange(len(tgt_s)) - starts[tgt_s]
    K2 = int(counts.max())
    planes = np.full((K2, N), PAD, np.int32)
    planes[rank, tgt_s] = src_s
    return planes, counts


def _wrap_idx(ix):
    """[n] int -> [128, n//16] int16 (wrapped in 16 partitions, replicated x8)."""
    return np.tile(ix.astype(np.int16).reshape(-1, 16).T, (8, 1))


def _prep(ref, nnf_sr, nnf_rs, f_a):
    """Host preprocessing. Returns (plan, per-core input maps skeleton)."""
    idxA = _build_pass1_planes(np.asarray(nnf_sr))
    idxB, c2 = _build_pass2_planes(np.asarray(nnf_rs))
    K2 = idxB.shape[0]

    refT = np.ascontiguousarray(np.asarray(ref).reshape(C, N).T)     # [N, C]
    faT = np.asarray(f_a).reshape(C, N).T                            # [N, C] (view)

    gorder = np.argsort(-c2, kind="stable")         # global rank -> target
    cores = []
    for c in range(NCORES):
        glob = gorder[c::NCORES]                    # local rank -> global target
        c2c = c2[glob]                              # descending by construction
        n_k = np.array([(c2c > k).sum() for k in range(K2)], np.int64)

        srcA = idxA[:, glob]                        # [9, TPC] global rows or PAD
        srcB = idxB[:, glob]                        # [K2, TPC]

        def compact_first_use(srcM):
            # order table rows by first use in the (k-major) gather stream so
            # consecutive descriptors hit nearby HBM rows
            stream = srcM.ravel()
            valid = stream != PAD
            vals, firsts = np.unique(stream[valid], return_index=True)
            uniq = vals[np.argsort(firsts)]
            lut = np.full(N, PAD, np.int32)
            lut[uniq] = np.arange(len(uniq), dtype=np.int32)
            rem = np.full(srcM.shape, PAD, np.int32)
            rem[srcM != PAD] = lut[srcM[srcM != PAD]]
            return uniq, rem

        uniqA, remA = compact_first_use(srcA)
        uniqB, remB = compact_first_use(srcB)
        # pads point at a zeros row appended to each table (index = n_uniq):
        # gathers write zeros -> accumulate is a no-op, and we avoid the
        # GatherAnt negative-index tail path which wedges the device.
        remA[remA == PAD] = len(uniqA)
        remB[remB == PAD] = len(uniqB)

        cores.append(dict(glob=glob, n_k=n_k, remA=remA, remB=remB,
                          uniqA=uniqA, uniqB=uniqB))

    VA = max(len(cc["uniqA"]) for cc in cores) + 1
    VB = max(len(cc["uniqB"]) for cc in cores) + 1
    assert VA <= 32767 and VB <= 32767, (VA, VB)
    n_k_max = np.max([cc["n_k"] for cc in cores], axis=0)            # [K2]

    # instruction plan shared by all cores: (series, slot_off, nslots)
    instrs = []
    for k in range(9):                      # series 1: full slabs
        for s0 in range(0, SLOTS, 8):
            ns = min(8, SLOTS - s0)
            instrs.append((1, k, s0, ns))
    for k in range(K2):                     # series 2: prefix slabs
        sk = int(np.ceil(n_k_max[k] / 128))
        if sk == 0:
            continue
        for s0 in range(0, sk, 8):
            ns = min(8, sk - s0)
            instrs.append((2, k, s0, ns))

    # per-core idx blobs + tables
    in_maps = []
    for cc in cores:
        blobs = []
        for (series, k, s0, ns) in instrs:
            rem = cc["remA"] if series == 1 else cc["remB"]
            zrow = len(cc["uniqA"]) if series == 1 else len(cc["uniqB"])
            rr = np.arange(s0 * 128, (s0 + ns) * 128)
            ix = rem[k][rr] if k < rem.shape[0] else np.full(ns * 128, zrow, np.int32)
            blobs.append(_wrap_idx(ix))
        idx_blob = np.ascontiguousarray(np.concatenate(blobs, axis=1))

        TA = np.zeros((VA, C), np.float32)
        TA[:len(cc["uniqA"])] = refT[cc["uniqA"]]
        TB = np.zeros((VB, C), np.float32)
        TB[:len(cc["uniqB"])] = refT[cc["uniqB"]]

        fa_core = faT[cc["glob"]]                                    # [TPC, C]
        fa_blob = np.ascontiguousarray(
            fa_core.reshape(SLOTS, 128, C).transpose(1, 0, 2)).reshape(128, SLOTS * C)

        c1 = (cc["remA"] != len(cc["uniqA"])).sum(axis=0)
        c2r = (cc["remB"] != len(cc["uniqB"])).sum(axis=0)
        den = (c1 + 2 * c2r).astype(np.float32)
        winv = np.where(den == 0, np.float32(1.0), 1.0 / np.maximum(den, 1)).astype(np.float32)
        winv_blob = np.ascontiguousarray(winv.reshape(SLOTS, 128).T)  # [128, SLOTS]

        in_maps.append({"ta": TA, "tb": TB, "idx": idx_blob.astype(np.int16),
                        "fa": fa_blob.astype(np.float32), "winv": winv_blob})

    plan = dict(instrs=instrs, VA=VA, VB=VB,
                W_total=sum(ns * 8 for (_, _, _, ns) in instrs))
    return plan, in_maps, cores


def _build_program(plan):
    from concourse import bacc, bass, mybir, tile

    VA, VB = plan["VA"], plan["VB"]
    WT = plan["W_total"]
    nc = bacc.Bacc("TRN2", target_bir_lowering=False, debug=False,
                   num_devices=NCORES)
    dt = mybir.dt
    ta = nc.dram_tensor("ta", [VA, C], dt.float32, kind="ExternalInput").ap()
    tb = nc.dram_tensor("tb", [VB, C], dt.float32, kind="ExternalInput").ap()
    idx = nc.dram_tensor("idx", [128, WT], dt.int16, kind="ExternalInput").ap()
    fa = nc.dram_tensor("fa", [128, SLOTS * C], dt.float32, kind="ExternalInput").ap()
    winv = nc.dram_tensor("winv", [128, SLOTS], dt.float32, kind="ExternalInput").ap()
    out = nc.dram_tensor("out", [128, SLOTS * C], dt.float32, kind="ExternalOutput").ap()

    with tile.TileContext(nc) as tc:
        with tc.tile_pool(name="sbuf", bufs=1) as pool, \
             tc.tile_pool(name="stg", bufs=3) as stp, \
             tc.tile_pool(name="fac", bufs=2) as fap, \
             tc.tile_pool(name="dram", bufs=1, space="DRAM") as dpool, \
             tc.tile_pool(name="psum", bufs=1, space="PSUM") as psp:
            idx_sb = pool.tile([128, WT], dt.int16)
            acc = pool.tile([128, SLOTS, C], dt.float32)
            winv_sb = pool.tile([128, SLOTS], dt.float32)
            resp = pool.tile([128, SLOTS], dt.float32)
            wt = pool.tile([128, SLOTS], dt.float32)
            sfac = pool.tile([128, SLOTS], dt.float32)
            red1 = pool.tile([128, 2], dt.float32)
            redc = pool.tile([1, 4], dt.float32)
            thrb = pool.tile([128, 2], dt.float32)
            thresh = pool.tile([128, 1], dt.float32)

            nc.sync.dma_start(out=idx_sb[:], in_=idx[:])
            nc.sync.dma_start(out=winv_sb[:], in_=winv[:])
            nc.vector.memset(acc[:], 0.0)
            ones1 = pool.tile([1, 128], dt.float32)
            nc.vector.memset(ones1[:], 1.0)

            # ---- response = sum_c fa^2, in CHUNK-slot chunks ----
            for ci in range(0, SLOTS, CHUNK):
                nch = min(CHUNK, SLOTS - ci)
                fch = fap.tile([128, CHUNK, C], dt.float32, tag="fch")
                sq = fap.tile([128, CHUNK, C], dt.float32, tag="sq")
                nc.sync.dma_start(out=fch[:, :nch, :],
                                  in_=fa[:, ci * C:(ci + nch) * C])
                nc.vector.tensor_mul(sq[:, :nch, :], fch[:, :nch, :], fch[:, :nch, :])
                nc.vector.tensor_reduce(resp[:, ci:ci + nch], sq[:, :nch, :],
                                        mybir.AxisListType.X, mybir.AluOpType.add)

            flat = pool.tile([1, 256], dt.float32)
            packv = pool.tile([1, 2], dt.float32)

            def emit_thresh_block():
                # cross-partition min/max: DMA-flatten [128,2] -> [1,256],
                # then one DVE max-reduce (red1[:,1] is already negated min)
                nc.vector.tensor_reduce(red1[:, 0:1], resp[:],
                                        mybir.AxisListType.X, mybir.AluOpType.max)
                nc.vector.tensor_reduce(red1[:, 1:2], resp[:],
                                        mybir.AxisListType.X, mybir.AluOpType.min)
                nc.vector.tensor_scalar_mul(red1[:, 1:2], red1[:, 1:2], -1.0)
                nc.sync.dma_start(out=flat[:], in_=red1[:])
                nc.vector.tensor_reduce(
                    packv[:], flat[:].rearrange("p (k j) -> p j k", j=2),
                    mybir.AxisListType.X, mybir.AluOpType.max)
                thr2 = pool.tile([1, 2], dt.float32)
                if _knob("NO_COLLECTIVE"):
                    nc.vector.tensor_copy(thr2[:], packv[:])
                else:
                    cc_in = dpool.tile([1, 2], dt.float32)
                    cc_out = dpool.tile([1, 2], dt.float32)
                    nc.sync.dma_start(out=cc_in[:], in_=packv[:])
                    nc.gpsimd.collective_compute(
                        "AllReduce", mybir.AluOpType.max,
                        replica_groups=[list(range(NCORES))],
                        ins=[cc_in.opt()], outs=[cc_out.opt()])
                    nc.sync.dma_start(out=thr2[:], in_=cc_out[:])
                # broadcast [1,2] -> [128,2] via PE ones-matmul (gpsimd queue
                # is the critical path; TensorE is idle)
                thr_ps = psp.tile([128, 2], dt.float32, space="PSUM")
                nc.tensor.matmul(out=thr_ps[:], lhsT=ones1[:], rhs=thr2[:],
                                 start=True, stop=True)
                nc.vector.tensor_copy(thrb[:], thr_ps[:])
                tmp1 = pool.tile([128, 1], dt.float32)
                nc.vector.tensor_scalar_mul(tmp1[:], thrb[:, 0:1], TAU)
                nc.vector.scalar_tensor_tensor(
                    out=thresh[:], in0=thrb[:, 1:2], scalar=-(1.0 - TAU), in1=tmp1[:],
                    op0=mybir.AluOpType.mult, op1=mybir.AluOpType.add)
                nc.vector.tensor_tensor(wt[:], resp[:],
                                        thresh[:].to_broadcast([128, SLOTS]),
                                        mybir.AluOpType.is_gt)
                nc.vector.tensor_scalar_mul(wt[:], wt[:], ALPHA)
                tmp2 = pool.tile([128, SLOTS], dt.float32)
                nc.vector.tensor_scalar(tmp2[:], wt[:], -1.0, 1.0,
                                        mybir.AluOpType.mult, mybir.AluOpType.add)
                nc.vector.tensor_tensor(sfac[:], tmp2[:], winv_sb[:],
                                        mybir.AluOpType.mult)

            # ---- gather + accumulate pipeline ----
            ninstr_cap = int(os.environ.get("BNNF_MAX_GATHERS", "999999"))
            woff = 0
            for gi, (series, k, s0, ns) in enumerate(plan["instrs"]):
                if gi >= ninstr_cap:
                    break
                nidx = ns * 128
                wcols = ns * 8
                stg = stp.tile([128, 8, C], dt.float32, tag="stage")
                src = ta if series == 1 else tb
                nc.gpsimd.dma_gather(
                    out_ap=stg[:, :ns, :], in_ap=src,
                    idxs_ap=idx_sb[:, woff:woff + wcols],
                    num_idxs=nidx, num_idxs_reg=nidx, elem_size=C, single_packet=False)
                aslice = acc[:, s0:s0 + ns, :]
                if series == 1:
                    nc.vector.tensor_add(aslice, aslice, stg[:, :ns, :])
                else:
                    nc.vector.scalar_tensor_tensor(
                        out=aslice, in0=stg[:, :ns, :], scalar=2.0, in1=aslice,
                        op0=mybir.AluOpType.mult, op1=mybir.AluOpType.add)
                woff += wcols
                if gi == 30:
                    emit_thresh_block()

            if ninstr_cap <= 30 or len(plan["instrs"]) <= 30:
                emit_thresh_block()
            # ---- blend: out = fa*w + acc*sfac ----
            for ci in range(0, SLOTS, CHUNK):
                nch = min(CHUNK, SLOTS - ci)
                fch = fap.tile([128, CHUNK, C], dt.float32, tag="fch2")
                nc.sync.dma_start(out=fch[:, :nch, :],
                                  in_=fa[:, ci * C:(ci + nch) * C])
                w_b = wt[:, ci:ci + nch].unsqueeze(2).to_broadcast([128, nch, C])
                s_b = sfac[:, ci:ci + nch].unsqueeze(2).to_broadcast([128, nch, C])
                ach = acc[:, ci:ci + nch, :]
                nc.vector.tensor_tensor(fch[:, :nch, :], fch[:, :nch, :], w_b,
                                        mybir.AluOpType.mult)
                nc.vector.tensor_tensor(ach, ach, s_b, mybir.AluOpType.mult)
                nc.vector.tensor_add(fch[:, :nch, :], fch[:, :nch, :], ach)
                nc.sync.dma_start(out=out[:, ci * C:(ci + nch) * C],
                                  in_=fch[:, :nch, :])
    nc.compile()
    return nc


def _install_ntff_hook():
    try:
        import antenv
        if "antenv.axon_hooks" not in sys.modules:
            mod = types.ModuleType("antenv.axon_hooks")
            _h = [None]
            mod.set_axon_ntff_profile_hook = lambda h: _h.__setitem__(0, h)
            mod.get_axon_ntff_profile_hook = lambda: _h[0]
            sys.modules["antenv.axon_hooks"] = mod
            antenv.axon_hooks = mod
            from trn_agent_boot.trn_boot import _ntff_profile_via_ctypes
            hook = _ntff_profile_via_ctypes('/opt/axon/libaxon_pjrt.so')
            if hook is not None:
                mod.set_axon_ntff_profile_hook(hook)
    except Exception:
        pass


def kernel(ref, f_a, nnf_sr, nnf_rs, _trace=False):
    from concourse.bass_utils import run_bass_kernel_spmd

    _install_ntff_hook()
    ref = np.asarray(ref)
    f_a = np.asarray(f_a)
    plan, in_maps, cores = _prep(ref, nnf_sr, nnf_rs, f_a)

    key = (plan["VA"], plan["VB"], plan["W_total"], tuple(plan["instrs"]),
           os.environ.get("BNNF_NO_COLLECTIVE", ""), os.environ.get("BNNF_MAX_GATHERS", ""))
    if _D.get("key") != key:
        _D["nc"] = _build_program(plan)
        _D["key"] = key
    nc = _D["nc"]

    res = run_bass_kernel_spmd(nc, in_maps, list(range(NCORES)), trace=_trace)
    if _trace:
        _D["exec_time_ns"] = res.exec_time_ns

    outT = np.empty((N, C), np.float32)
    for c, cc in enumerate(cores):
        blob = res.results[c]["out"].reshape(128, SLOTS, C)
        core_rows = blob.transpose(1, 0, 2).reshape(TPC, C)   # rank -> row
        outT[cc["glob"]] = core_rows
    return np.ascontiguousarray(outT.T).reshape(1, C, H, W).astype(np.float32)
